# revision 20
# baseline (speedup 1.0000x reference)
"""Binary linear layer (sign(X) @ sign(W) * scale) on 8 trn2 NeuronCores.

Strategy: data-parallel over the batch dim. Each core gets 1/8 of X's rows.
Host-side prep binarizes X and W to +-1 fp8e4m3 (sign-exact, so device math
is bit-identical to sign()@sign()) and packs k-row PAIRS per DRAM row so
every DMA descriptor is 2048B. On-device: whole-K fp8 tiles SBUF-resident,
DoubleRow fp8 matmuls (0.5 cyc/row, the TRN2 PE peak) accumulating straight
in PSUM, a fused scale-multiply eviction split across DVE/GpSimd, f32 out.

Schedule notes (from perfetto traces of prior versions):
- A matmul whose lhsT (stationary) differs from its predecessor costs
  ~259ns; with the same lhsT it streams at ~213ns. So all groups after the
  first use (2 m-tiles x 3 n-cols) PSUM residency -> lhsT shared across 3
  consecutive MMs. Group 0 must be n-narrow (8mt x 1nt) because its DMA
  first-touch (X + W-half0) paces it.
- The PE clock ramps 0.65 -> 1.2 -> 2.4 GHz after 3us of CONTINUOUS busy;
  any gap drops it back. Junk warmup MMs pre-ramp it while the first
  chunks land, and tiny junk pads at group boundaries absorb the
  eviction-latency bubble so the clock never resets.
- Everything input rides the sync HWDGE queue (fine-grained completion
  sems). The scale chain lives on gpsimd (partition_broadcast instead of a
  log-doubling DMA chain); Y stores ride the idle ACT engine's queue.
"""

import os

import numpy as np

import concourse.bacc as bacc
import concourse.mybir as mybir
from concourse.tile import TileContext

P = 128
B, IN, OUT = 8192, 4096, 2048
NCORES = 8
M = B // NCORES  # 1024 rows per core
K = IN
N = OUT
NPAIR = K // (2 * P)  # 16 k-pair blocks; block i covers k = [i*256, (i+1)*256)
NMT = M // P  # 8 m-tiles
FD = 512  # psum tile free dim (one bank)
NNT = N // FD  # 4 n-cols


def build_kernel(
    warmup_mms=6,  # junk PE matmuls at t=0: pre-ramp the HAM clock gate
    mid_junk=3,  # junk MMs between scale outer-MM and broadcast MMs
    bjunk=4,  # junk MMs at group boundaries: absorb eviction latency
    out_bufs=6,
    ev_gpsimd=False,  # alternate evictions DVE/GpSimd
    wh1_scalar=True,  # W half-1 loads ride the scalar HWDGE queue
):
    f32 = mybir.dt.float32
    fp8 = mybir.dt.float8e4
    pm = mybir.MatmulPerfMode.DoubleRow
    AF = mybir.ActivationFunctionType

    nc = bacc.Bacc("TRN2", debug=False, num_devices=NCORES)

    # XP[i*128+p, s*1024+m] = sign(X)[m-th row of this core's slice, k]
    # with k = 2*(i*128+p)+s  (pair-packed; see _make_in_maps)
    XP = nc.declare_dram_parameter("XP", [NPAIR * P, 2 * M], fp8, isOutput=False)
    # WH[h*2048 + i*128 + p, s*1024 + c] = sign(W)[k, h*1024 + c], same k map
    WH = nc.declare_dram_parameter("WH", [2 * NPAIR * P, N], fp8, isOutput=False)
    alpha = nc.declare_dram_parameter("alpha", [1], f32, isOutput=False)
    betta = nc.declare_dram_parameter("betta", [32], f32, isOutput=False)
    gamma = nc.declare_dram_parameter("gamma", [64], f32, isOutput=False)
    SCR = nc.declare_dram_parameter("SCR", [1, N], f32, isOutput=False)
    Y = nc.declare_dram_parameter("Y", [M, N], f32, isOutput=True)

    with TileContext(nc) as tc:
        with (
            tc.tile_pool(name="const", bufs=1) as cpool,
            tc.tile_pool(name="bin", bufs=2) as binpool,
            tc.tile_pool(name="outp", bufs=2) as outpool,
            tc.tile_pool(name="psum", bufs=8, space="PSUM") as pspool,
        ):
            scale_t = cpool.tile([P, N], f32, bufs=1)

            # ---- PE warm-up: no-dep junk matmuls fill the clock-ramp window
            # while the first input chunks are still in flight ----
            wu = cpool.tile([P, 2, 256], fp8, bufs=1)
            nc.vector.memset(wu, 0)
            ps_wu = pspool.tile([P, FD], f32, tag="mm", bufs=8)

            def emit_junk(n):
                for _ in range(n):
                    nc.tensor.matmul(
                        ps_wu[:, :256],
                        lhsT=wu[:, :, :P],
                        rhs=wu,
                        start=True,
                        stop=True,
                        perf_mode=pm,
                    )

            emit_junk(warmup_mms)

            # ---- scale grid: relu(alpha) * outer(relu(betta), relu(gamma)),
            # then broadcast to all 128 partitions with 4 tiny PE matmuls
            # (ones[1,128].T @ sc_row[1,512]) — rides the PE's idle
            # wait-for-first-chunk window. ----
            a_t = cpool.tile([1, 1], f32, bufs=1)
            b_t = cpool.tile([1, 32], f32, bufs=1)
            g_t = cpool.tile([1, 64], f32, bufs=1)
            # sync queue head: gens fire ~2us before gpsimd's, so the relu ->
            # outer-MM chain completes before the warmup junk runs out
            nc.sync.dma_start(out=a_t, in_=alpha[:])
            nc.sync.dma_start(out=b_t, in_=betta[:])
            nc.sync.dma_start(out=g_t, in_=gamma[:])
            nc.scalar.activation(a_t, a_t, AF.Relu)
            # relu(betta)*relu(alpha) == relu(betta*relu(alpha)) since relu(alpha)>=0
            b_s = cpool.tile([1, 32], f32, bufs=1)
            g_r = cpool.tile([1, 64], f32, bufs=1)
            nc.scalar.activation(b_s, b_t, AF.Relu, scale=a_t[0:1, 0:1])
            nc.scalar.activation(g_r, g_t, AF.Relu)
            ps_sc = pspool.tile([P, FD], f32, tag="mm", bufs=8)
            nc.tensor.matmul(ps_sc[:32, :64], lhsT=b_s, rhs=g_r, start=True, stop=True)
            emit_junk(mid_junk)
            # flatten [32,64] via DRAM bounce (row-major == n = i*64+j), then
            # ONE broadcast-read DMA fans the 8KB row to all 128 partitions.
            # Two ~5us DMA hops — a log-doubling SBUF chain costs 7 serial
            # hops (~38us) and starved the evictions in earlier versions.
            sc_tmp = cpool.tile([32, 64], f32, bufs=1)
            nc.vector.tensor_copy(out=sc_tmp, in_=ps_sc[:32, :64])
            nc.gpsimd.dma_start(out=SCR[0:1, :], in_=sc_tmp)
            nc.gpsimd.dma_start(out=scale_t, in_=SCR[0:1, :].to_broadcast((P, N)))

            # ---- input loads (all sync/HWDGE): X + W-half0 interleaved
            # (group 0's diet), then W-half1 ----
            xbs = []
            wbs = [[None] * NPAIR, [None] * NPAIR]
            for i in range(NPAIR):
                xb = binpool.tile([P, 2, M], fp8, tag="xb", bufs=NPAIR)
                nc.sync.dma_start(out=xb, in_=XP[i * P : (i + 1) * P, :])
                xbs.append(xb)
                wb = binpool.tile([P, 2, N // 2], fp8, tag="wb", bufs=2 * NPAIR)
                nc.sync.dma_start(out=wb, in_=WH[i * P : (i + 1) * P, :])
                wbs[0][i] = wb
            wq = nc.scalar if wh1_scalar else nc.sync
            for i in range(NPAIR):
                wb = binpool.tile([P, 2, N // 2], fp8, tag="wb", bufs=2 * NPAIR)
                wq.dma_start(
                    out=wb, in_=WH[(NPAIR + i) * P : (NPAIR + i + 1) * P, :]
                )
                wbs[1][i] = wb

            def rhs_of(i, nt):
                h, sub = divmod(nt, 2)
                return wbs[h][i][:, :, sub * FD : (sub + 1) * FD]

            def emit_evs(banks):
                # banks: list of (ps, mt, nt); alternate DVE / GpSimd
                for j, (ps, mt, nt) in enumerate(banks):
                    ot = outpool.tile([P, FD], f32, tag="ot", bufs=out_bufs)
                    eng = nc.gpsimd if (ev_gpsimd and j % 2 == 1) else nc.vector
                    eng.tensor_mul(
                        out=ot, in0=ps, in1=scale_t[:, nt * FD : (nt + 1) * FD]
                    )
                    nc.scalar.dma_start(
                        out=Y[mt * P : (mt + 1) * P, nt * FD : (nt + 1) * FD],
                        in_=ot,
                    )

            def emit_bjunk():
                if not bjunk:
                    return
                ps_j = pspool.tile([P, FD], f32, tag="mm", bufs=8, name="ps_j")
                for _ in range(bjunk):
                    nc.tensor.matmul(
                        ps_j[:, :256],
                        lhsT=wu[:, :, :P],
                        rhs=wu,
                        start=True,
                        stop=True,
                        perf_mode=pm,
                    )

            # ---- group 0: (8 m-tiles x n-col 0). n-narrow so the sync queue
            # outruns the PE; pays the unique-lhsT matmul rate. ----
            pss = [
                pspool.tile([P, FD], f32, tag="mm", bufs=8, name=f"ps0_{mt}")
                for mt in range(NMT)
            ]
            for i in range(NPAIR):
                rhs = rhs_of(i, 0)
                for mt in range(NMT):
                    nc.tensor.matmul(
                        pss[mt],
                        lhsT=xbs[i][:, :, mt * P : (mt + 1) * P],
                        rhs=rhs,
                        start=(i == 0),
                        stop=(i == NPAIR - 1),
                        perf_mode=pm,
                    )
            emit_bjunk()
            emit_evs([(pss[mt], mt, 0) for mt in range(NMT)])

            # ---- groups 1+: (m-tiles x n-cols 1-3): lhsT shared across 3
            # consecutive MMs. The last two groups are single-m-tile so the
            # final serial DVE eviction chain is 3 muls, not 6. ----
            mt_groups = [(0, 1), (2, 3), (4, 5), (6,), (7,)]
            for g, mts in enumerate(mt_groups):
                banks = {}
                for mt in mts:
                    for nt in (1, 2, 3):
                        banks[(mt, nt)] = pspool.tile(
                            [P, FD], f32, tag="mm", bufs=8, name=f"ps{g + 1}_{mt}_{nt}"
                        )
                for i in range(NPAIR):
                    for mt in mts:
                        lhsT = xbs[i][:, :, mt * P : (mt + 1) * P]
                        for nt in (1, 2, 3):
                            nc.tensor.matmul(
                                banks[(mt, nt)],
                                lhsT=lhsT,
                                rhs=rhs_of(i, nt),
                                start=(i == 0),
                                stop=(i == NPAIR - 1),
                                perf_mode=pm,
                            )
                if g < len(mt_groups) - 1:
                    emit_bjunk()
                emit_evs([(ps, mt, nt) for (mt, nt), ps in banks.items()])
    return nc


_NC_CACHE = {}


def _get_nc(**kw):
    key = tuple(sorted(kw.items()))
    if key not in _NC_CACHE:
        nc = build_kernel(**kw)
        nc.finalize()
        _NC_CACHE[key] = nc
    return _NC_CACHE[key]


def _make_in_maps(X, W, alpha, betta, gamma):
    fp8 = mybir.dt.np(mybir.dt.float8e4)
    X = np.asarray(X, dtype=np.float32)
    W = np.asarray(W, dtype=np.float32)
    # +-1 is exact in fp8e4m3, so the device matmul is bit-identical to
    # sign(X) @ sign(W)
    Wb = np.sign(W).astype(fp8)  # [K, N]
    # WH[h*2048+r, s*1024+c] = Wb[2r+s, h*1024+c]: pair-packed 2048B rows,
    # halved along N so groups 0-1's half can be DMA'd ahead
    WH = np.ascontiguousarray(
        Wb.reshape(K // 2, 2, 2, N // 2).transpose(2, 0, 1, 3)
    ).reshape(K, N)
    alpha = np.asarray(alpha, dtype=np.float32).reshape([1])
    betta = np.asarray(betta, dtype=np.float32).reshape([32])
    gamma = np.asarray(gamma, dtype=np.float32).reshape([64])
    scr = np.zeros((1, N), dtype=np.float32)  # device-side scale scratch
    in_maps = []
    for c in range(NCORES):
        xs = np.sign(X[c * M : (c + 1) * M, :]).astype(fp8)  # [M, K]
        # XP[r, s*1024+m] = sign(X).T[2r+s, m] — a contiguous k-major
        # transpose pair-packs rows for free
        xp = np.ascontiguousarray(xs.T).reshape(K // 2, 2 * M)
        in_maps.append(
            {
                "XP": xp,
                "WH": WH,
                "alpha": alpha,
                "betta": betta,
                "gamma": gamma,
                "SCR": scr,
            }
        )
    return in_maps


def run_on_cores(inputs, trace=False, tmpdir=None, **build_kw):
    """Run the SPMD kernel on 8 cores; returns (Y_full, BassKernelResults)."""
    from concourse.bass_utils import run_bass_kernel_spmd

    if not trace:
        # this image lacks antenv.axon_hooks; a stray BASS_TRACE env var would
        # crash run_bass_kernel_spmd's trace branch, so fail safe
        try:
            import antenv.axon_hooks  # noqa: F401
        except ImportError:
            os.environ.setdefault("BASS_NEVER_TRACE", "1")
    nc = _get_nc(**build_kw)
    in_maps = _make_in_maps(**inputs)
    res = run_bass_kernel_spmd(
        nc, in_maps, list(range(NCORES)), trace=trace, tmpdir=tmpdir
    )
    Yf = np.concatenate([r["Y"] for r in res.results], axis=0)
    return Yf, res


PROD_KW = dict(
    warmup_mms=20,
    mid_junk=2,
    bjunk=3,
    out_bufs=6,
    ev_gpsimd=False,  # GpSimd ALU ops need a ucode library this backend lacks
    wh1_scalar=True,
)


def kernel(**inputs) -> np.ndarray:
    Yr, _ = run_on_cores(inputs, **PROD_KW)
    return Yr


# revision 22
# speedup vs baseline: 1.0194x; 1.0194x over previous
"""Binary linear layer (sign(X) @ sign(W) * scale) on 8 trn2 NeuronCores.

Strategy: data-parallel over the batch dim. Each core gets 1/8 of X's rows.
Host-side prep binarizes X and W to +-1 fp8e4m3 (sign-exact, so device math
is bit-identical to sign()@sign()) and packs k-row PAIRS per DRAM row so
every DMA descriptor is 2048B. On-device: whole-K fp8 tiles SBUF-resident,
DoubleRow fp8 matmuls (0.5 cyc/row, the TRN2 PE peak) accumulating straight
in PSUM, a fused scale-multiply eviction split across DVE/GpSimd, f32 out.

Schedule notes (from perfetto traces of prior versions):
- A matmul whose lhsT (stationary) differs from its predecessor costs
  ~259ns; with the same lhsT it streams at ~213ns. So all groups after the
  first use (2 m-tiles x 3 n-cols) PSUM residency -> lhsT shared across 3
  consecutive MMs. Group 0 must be n-narrow (8mt x 1nt) because its DMA
  first-touch (X + W-half0) paces it.
- The PE clock ramps 0.65 -> 1.2 -> 2.4 GHz after 3us of CONTINUOUS busy;
  any gap drops it back. Junk warmup MMs pre-ramp it while the first
  chunks land, and tiny junk pads at group boundaries absorb the
  eviction-latency bubble so the clock never resets.
- Everything input rides the sync HWDGE queue (fine-grained completion
  sems). The scale chain lives on gpsimd (partition_broadcast instead of a
  log-doubling DMA chain); Y stores ride the idle ACT engine's queue.
"""

import os

import numpy as np

import concourse.bacc as bacc
import concourse.mybir as mybir
from concourse.tile import TileContext

P = 128
B, IN, OUT = 8192, 4096, 2048
NCORES = 8
M = B // NCORES  # 1024 rows per core
K = IN
N = OUT
NPAIR = K // (2 * P)  # 16 k-pair blocks; block i covers k = [i*256, (i+1)*256)
NMT = M // P  # 8 m-tiles
FD = 512  # psum tile free dim (one bank)
NNT = N // FD  # 4 n-cols


def build_kernel(
    warmup_mms=6,  # junk PE matmuls at t=0: pre-ramp the HAM clock gate
    mid_junk=3,  # junk MMs between scale outer-MM and broadcast MMs
    bjunk=4,  # junk MMs at group boundaries: absorb eviction latency
    out_bufs=6,
    ev_gpsimd=False,  # alternate evictions DVE/GpSimd
    wh1_scalar=True,  # W half-1 loads ride the scalar HWDGE queue
):
    f32 = mybir.dt.float32
    fp8 = mybir.dt.float8e4
    pm = mybir.MatmulPerfMode.DoubleRow
    AF = mybir.ActivationFunctionType

    nc = bacc.Bacc("TRN2", debug=False, num_devices=NCORES)

    # XP[i*128+p, s*1024+m] = sign(X)[m-th row of this core's slice, k]
    # with k = 2*(i*128+p)+s  (pair-packed; see _make_in_maps)
    XP = nc.declare_dram_parameter("XP", [NPAIR * P, 2 * M], fp8, isOutput=False)
    # WH[h*2048 + i*128 + p, s*1024 + c] = sign(W)[k, h*1024 + c], same k map
    WH = nc.declare_dram_parameter("WH", [2 * NPAIR * P, N], fp8, isOutput=False)
    alpha = nc.declare_dram_parameter("alpha", [1], f32, isOutput=False)
    betta = nc.declare_dram_parameter("betta", [32], f32, isOutput=False)
    gamma = nc.declare_dram_parameter("gamma", [64], f32, isOutput=False)
    SCR = nc.declare_dram_parameter("SCR", [1, N], f32, isOutput=False)
    Y = nc.declare_dram_parameter("Y", [M, N], f32, isOutput=True)

    with TileContext(nc) as tc:
        with (
            tc.tile_pool(name="const", bufs=1) as cpool,
            tc.tile_pool(name="bin", bufs=2) as binpool,
            tc.tile_pool(name="outp", bufs=2) as outpool,
            tc.tile_pool(name="psum", bufs=8, space="PSUM") as pspool,
        ):
            scale_t = cpool.tile([P, N], f32, bufs=1)

            # ---- PE warm-up: no-dep junk matmuls fill the clock-ramp window
            # while the first input chunks are still in flight ----
            wu = cpool.tile([P, 2, 256], fp8, bufs=1)
            nc.vector.memset(wu, 0)
            ps_wu = pspool.tile([P, FD], f32, tag="mm", bufs=8)

            def emit_junk(n):
                for _ in range(n):
                    nc.tensor.matmul(
                        ps_wu[:, :256],
                        lhsT=wu[:, :, :P],
                        rhs=wu,
                        start=True,
                        stop=True,
                        perf_mode=pm,
                    )

            emit_junk(warmup_mms)

            # ---- scale grid: relu(alpha) * outer(relu(betta), relu(gamma)),
            # then broadcast to all 128 partitions with 4 tiny PE matmuls
            # (ones[1,128].T @ sc_row[1,512]) — rides the PE's idle
            # wait-for-first-chunk window. ----
            a_t = cpool.tile([1, 1], f32, bufs=1)
            b_t = cpool.tile([1, 32], f32, bufs=1)
            g_t = cpool.tile([1, 64], f32, bufs=1)
            nc.gpsimd.dma_start(out=a_t, in_=alpha[:])
            nc.gpsimd.dma_start(out=b_t, in_=betta[:])
            nc.gpsimd.dma_start(out=g_t, in_=gamma[:])
            nc.scalar.activation(a_t, a_t, AF.Relu)
            # relu(betta)*relu(alpha) == relu(betta*relu(alpha)) since relu(alpha)>=0
            b_s = cpool.tile([1, 32], f32, bufs=1)
            g_r = cpool.tile([1, 64], f32, bufs=1)
            nc.scalar.activation(b_s, b_t, AF.Relu, scale=a_t[0:1, 0:1])
            nc.scalar.activation(g_r, g_t, AF.Relu)
            ps_sc = pspool.tile([P, FD], f32, tag="mm", bufs=8)
            nc.tensor.matmul(ps_sc[:32, :64], lhsT=b_s, rhs=g_r, start=True, stop=True)
            emit_junk(mid_junk)
            # flatten [32,64] via DRAM bounce (row-major == n = i*64+j), then
            # ONE broadcast-read DMA fans the 8KB row to all 128 partitions.
            # Two ~5us DMA hops — a log-doubling SBUF chain costs 7 serial
            # hops (~38us) and starved the evictions in earlier versions.
            sc_tmp = cpool.tile([32, 64], f32, bufs=1)
            nc.vector.tensor_copy(out=sc_tmp, in_=ps_sc[:32, :64])
            nc.gpsimd.dma_start(out=SCR[0:1, :], in_=sc_tmp)
            nc.gpsimd.dma_start(out=scale_t, in_=SCR[0:1, :].to_broadcast((P, N)))

            # ---- input loads (all sync/HWDGE): X + W-half0 interleaved
            # (group 0's diet), then W-half1 ----
            xbs = []
            wbs = [[None] * NPAIR, [None] * NPAIR]
            for i in range(NPAIR):
                xb = binpool.tile([P, 2, M], fp8, tag="xb", bufs=NPAIR)
                nc.sync.dma_start(out=xb, in_=XP[i * P : (i + 1) * P, :])
                xbs.append(xb)
                wb = binpool.tile([P, 2, N // 2], fp8, tag="wb", bufs=2 * NPAIR)
                nc.sync.dma_start(out=wb, in_=WH[i * P : (i + 1) * P, :])
                wbs[0][i] = wb
            wq = nc.scalar if wh1_scalar else nc.sync
            for i in range(NPAIR):
                wb = binpool.tile([P, 2, N // 2], fp8, tag="wb", bufs=2 * NPAIR)
                wq.dma_start(
                    out=wb, in_=WH[(NPAIR + i) * P : (NPAIR + i + 1) * P, :]
                )
                wbs[1][i] = wb

            def rhs_of(i, nt):
                h, sub = divmod(nt, 2)
                return wbs[h][i][:, :, sub * FD : (sub + 1) * FD]

            def emit_evs(banks):
                # banks: list of (ps, mt, nt); alternate DVE / GpSimd
                for j, (ps, mt, nt) in enumerate(banks):
                    ot = outpool.tile([P, FD], f32, tag="ot", bufs=out_bufs)
                    eng = nc.gpsimd if (ev_gpsimd and j % 2 == 1) else nc.vector
                    eng.tensor_mul(
                        out=ot, in0=ps, in1=scale_t[:, nt * FD : (nt + 1) * FD]
                    )
                    nc.scalar.dma_start(
                        out=Y[mt * P : (mt + 1) * P, nt * FD : (nt + 1) * FD],
                        in_=ot,
                    )

            def emit_bjunk():
                if not bjunk:
                    return
                ps_j = pspool.tile([P, FD], f32, tag="mm", bufs=8, name="ps_j")
                for _ in range(bjunk):
                    nc.tensor.matmul(
                        ps_j[:, :256],
                        lhsT=wu[:, :, :P],
                        rhs=wu,
                        start=True,
                        stop=True,
                        perf_mode=pm,
                    )

            # ---- group 0: (8 m-tiles x n-col 0). n-narrow so the sync queue
            # outruns the PE; pays the unique-lhsT matmul rate. ----
            pss = [
                pspool.tile([P, FD], f32, tag="mm", bufs=8, name=f"ps0_{mt}")
                for mt in range(NMT)
            ]
            for i in range(NPAIR):
                rhs = rhs_of(i, 0)
                for mt in range(NMT):
                    nc.tensor.matmul(
                        pss[mt],
                        lhsT=xbs[i][:, :, mt * P : (mt + 1) * P],
                        rhs=rhs,
                        start=(i == 0),
                        stop=(i == NPAIR - 1),
                        perf_mode=pm,
                    )
            emit_bjunk()
            emit_evs([(pss[mt], mt, 0) for mt in range(NMT)])

            # ---- groups 1+: (m-tiles x n-cols 1-3): lhsT shared across 3
            # consecutive MMs. The last two groups are single-m-tile so the
            # final serial DVE eviction chain is 3 muls, not 6. ----
            mt_groups = [(0, 1), (2, 3), (4, 5), (6,), (7,)]
            for g, mts in enumerate(mt_groups):
                banks = {}
                for mt in mts:
                    for nt in (1, 2, 3):
                        banks[(mt, nt)] = pspool.tile(
                            [P, FD], f32, tag="mm", bufs=8, name=f"ps{g + 1}_{mt}_{nt}"
                        )
                for i in range(NPAIR):
                    for mt in mts:
                        lhsT = xbs[i][:, :, mt * P : (mt + 1) * P]
                        for nt in (1, 2, 3):
                            nc.tensor.matmul(
                                banks[(mt, nt)],
                                lhsT=lhsT,
                                rhs=rhs_of(i, nt),
                                start=(i == 0),
                                stop=(i == NPAIR - 1),
                                perf_mode=pm,
                            )
                if g < len(mt_groups) - 1:
                    emit_bjunk()
                emit_evs([(ps, mt, nt) for (mt, nt), ps in banks.items()])
    return nc


_NC_CACHE = {}


def _get_nc(**kw):
    key = tuple(sorted(kw.items()))
    if key not in _NC_CACHE:
        nc = build_kernel(**kw)
        nc.finalize()
        _NC_CACHE[key] = nc
    return _NC_CACHE[key]


def _make_in_maps(X, W, alpha, betta, gamma):
    fp8 = mybir.dt.np(mybir.dt.float8e4)
    X = np.asarray(X, dtype=np.float32)
    W = np.asarray(W, dtype=np.float32)
    # +-1 is exact in fp8e4m3, so the device matmul is bit-identical to
    # sign(X) @ sign(W)
    Wb = np.sign(W).astype(fp8)  # [K, N]
    # WH[h*2048+r, s*1024+c] = Wb[2r+s, h*1024+c]: pair-packed 2048B rows,
    # halved along N so groups 0-1's half can be DMA'd ahead
    WH = np.ascontiguousarray(
        Wb.reshape(K // 2, 2, 2, N // 2).transpose(2, 0, 1, 3)
    ).reshape(K, N)
    alpha = np.asarray(alpha, dtype=np.float32).reshape([1])
    betta = np.asarray(betta, dtype=np.float32).reshape([32])
    gamma = np.asarray(gamma, dtype=np.float32).reshape([64])
    scr = np.zeros((1, N), dtype=np.float32)  # device-side scale scratch
    in_maps = []
    for c in range(NCORES):
        xs = np.sign(X[c * M : (c + 1) * M, :]).astype(fp8)  # [M, K]
        # XP[r, s*1024+m] = sign(X).T[2r+s, m] — a contiguous k-major
        # transpose pair-packs rows for free
        xp = np.ascontiguousarray(xs.T).reshape(K // 2, 2 * M)
        in_maps.append(
            {
                "XP": xp,
                "WH": WH,
                "alpha": alpha,
                "betta": betta,
                "gamma": gamma,
                "SCR": scr,
            }
        )
    return in_maps


def run_on_cores(inputs, trace=False, tmpdir=None, **build_kw):
    """Run the SPMD kernel on 8 cores; returns (Y_full, BassKernelResults)."""
    from concourse.bass_utils import run_bass_kernel_spmd

    if not trace:
        # this image lacks antenv.axon_hooks; a stray BASS_TRACE env var would
        # crash run_bass_kernel_spmd's trace branch, so fail safe
        try:
            import antenv.axon_hooks  # noqa: F401
        except ImportError:
            os.environ.setdefault("BASS_NEVER_TRACE", "1")
    nc = _get_nc(**build_kw)
    in_maps = _make_in_maps(**inputs)
    res = run_bass_kernel_spmd(
        nc, in_maps, list(range(NCORES)), trace=trace, tmpdir=tmpdir
    )
    Yf = np.concatenate([r["Y"] for r in res.results], axis=0)
    return Yf, res


PROD_KW = dict(
    warmup_mms=23,
    mid_junk=2,
    bjunk=3,
    out_bufs=6,
    ev_gpsimd=False,  # GpSimd ALU ops need a ucode library this backend lacks
    wh1_scalar=True,
)


def kernel(**inputs) -> np.ndarray:
    Yr, _ = run_on_cores(inputs, **PROD_KW)
    return Yr
